# revision 1
# baseline (speedup 1.0000x reference)
"""GPSA transformer block (ConViT-style) for TRN2, data-parallel over 8 cores.

Layout strategy: activations feature-major [C, T] (T = 8*196 tokens/core).
All matmuls bf16 inputs + fp32 PSUM accumulation, except tiny LN-stat /
bias-broadcast matmuls which run as float32r (fp32 bits).

Attention per (h, b): S[n,m] on PE -> exp(+row-sum accum) on ACT ->
(E*r + sQbar) fused on DVE -> PE-transpose -> PSUM->SBUF copy ->
O = vT.T @ St on PE -> scaled copy assembles O feature-major.
The softmax renorm `attn/sum(attn)` is folded into r = 1/den (row sums of the
combined attn are 1 +- 1e-6 so the explicit division is skipped; the (1-sig)
gate is applied via the O-copy scale and sQbar is pre-divided by (1-sig)).

SBUF pressure: x and x1 are only SBUF-resident while layernorm needs them;
the residual adds re-stream them (x from the input DRAM tensor, x1 from an
internal DRAM scratch written during phase 5).
"""
import sys

sys.path.insert(0, "/opt/trn_rl_repo")

import numpy as np
import ml_dtypes

import concourse.bass as bass
import concourse.mybir as mybir
from concourse import tile
from concourse.masks import make_identity

F32 = mybir.dt.float32
F32R = mybir.dt.float32r
BF16 = mybir.dt.bfloat16
AF = mybir.ActivationFunctionType
ALU = mybir.AluOpType

B, N, C, H = 64, 196, 768, 16
D = 48
DP = 64            # padded head dim
CP = H * DP        # 1024 padded attention channels
FF = 4 * C         # 3072
NCORES = 8
BLOC = B // NCORES  # 8 batches per core
T = BLOC * N        # 1568 tokens per core
TCH = 392           # token chunk (4 chunks; PSUM bank = 512 fp32)
NCHUNK = T // TCH   # 4
KC = C // 128       # 6 k-tiles over C
KCP = CP // 128     # 8 k-tiles over padded C'
KFF = FF // 128     # 24
SCALE = float(D) ** -0.5
EPS = 1e-5
NT = ((0, 128), (128, 68))  # token split within one batch


MAXW = 1  # walrus in this container rejects multi-wait instructions


class PatchedTileContext(tile.TileContext):
    """walrus in this container rejects >MAXW sync waits on one instruction
    ("Too many sync wait commands"). Split excess waits onto nofuse NoOps
    emitted just before the instruction on the same engine, and emit the
    exit-drain waits one per instruction."""

    def _add_instruction(self, inst):
        si = getattr(inst, "sync_info", None)
        waits = list(si.on_wait) if (si is not None and si.on_wait) else []
        if len(waits) > MAXW:
            extra = waits[:-MAXW]
            keep = waits[-MAXW:]
            for i in range(0, len(extra), MAXW):
                nop = mybir.InstNoOp(
                    name=f"{inst.name}_xw{i}",
                    sync_info=mybir.SyncInfo(
                        on_wait=extra[i : i + MAXW], on_update=[]),
                    bass_nofuse=True,
                    engine=inst.engine,
                )
                super()._add_instruction(nop)
            inst.sync_info = mybir.SyncInfo(
                on_wait=keep, on_update=list(si.on_update or []))
        super()._add_instruction(inst)

    def _drain_and_barrier(self, tick_clock, wait_clock):
        nc = self.nc
        clock = list(tick_clock.global_clock)
        for proc, sem in sorted(self.sems.allocated().items()):
            tick = clock[proc] if proc < len(clock) else 0
            if tick <= 0:
                continue
            mult = 16 if sem.name.startswith("DMA") else 1
            nc.sync.wait_ge(sem, tick * mult)
        nc.sync.drain()
        nc.all_engine_barrier()
        popped = nc._tile_sem_poison_stack.pop()
        assert popped is self._sem_poison
        nc.clear_and_free_semaphores(list(self.sems.allocated().values()))
        nc.all_engine_barrier()


def host_prep(inputs):
    """Preprocess full-problem numpy inputs into per-core DRAM tensor maps."""
    f32 = np.float32
    bf16 = ml_dtypes.bfloat16
    x = np.asarray(inputs["x"], f32)              # [B, N, C]
    qk_w = np.asarray(inputs["qk_w"], f32)        # [2C, C]
    v_w = np.asarray(inputs["v_w"], f32)          # [C, C]
    proj_w = np.asarray(inputs["proj_w"], f32)    # [C, C]
    fc1_w = np.asarray(inputs["fc1_w"], f32)      # [FF, C]
    fc2_w = np.asarray(inputs["fc2_w"], f32)      # [C, FF]

    # padded q/k lhsT: [C, 2*CP]; q head h -> cols 64h..64h+48, k -> CP + same
    wqk = np.zeros((C, 2 * CP), f32)
    for h in range(H):
        wqk[:, DP * h : DP * h + D] = qk_w[D * h : D * h + D, :].T
        wqk[:, CP + DP * h : CP + DP * h + D] = qk_w[C + D * h : C + D * h + D, :].T
    # padded v rhs: [C, CP] (token-major v output)
    wv = np.zeros((C, CP), f32)
    for h in range(H):
        wv[:, DP * h : DP * h + D] = v_w[D * h : D * h + D, :].T
    # padded proj lhsT: [CP, C]
    wproj = np.zeros((CP, C), f32)
    for h in range(H):
        wproj[DP * h : DP * h + D, :] = proj_w[:, D * h : D * h + D].T

    sig = 1.0 / (1.0 + np.exp(-np.asarray(inputs["gating"], np.float64)))  # [H]
    one_m_sig = (1.0 - sig).astype(f32)

    # pos attention, batch-independent: sQbar[h,n,m] = sig_h*softmax_m(.)/(1-sig_h)
    s = int(N ** 0.5)
    ind = np.arange(s)[None, :] - np.arange(s)[:, None]
    indx = np.tile(ind, (s, s))
    indy = np.repeat(np.repeat(ind, s, axis=0), s, axis=1)
    rel = np.stack([indx, indy, indx ** 2 + indy ** 2], -1).astype(f32)  # [N,N,3]
    logits = rel @ np.asarray(inputs["pos_w"], f32).T + np.asarray(inputs["pos_b"], f32)
    logits = logits.transpose(2, 0, 1).astype(np.float64)  # [H, N, N]
    e = np.exp(logits - logits.max(-1, keepdims=True))
    posP = e / e.sum(-1, keepdims=True)
    sqb = (posP * (sig / np.maximum(1.0 - sig, 1e-20))[:, None, None]).astype(f32)
    sqb = np.ascontiguousarray(sqb.transpose(0, 2, 1))  # [h, m, n] (pre-transposed)

    common = {
        "wqk": wqk.astype(bf16),
        "wv": wv.astype(bf16),
        "wproj": wproj.astype(bf16),
        "wfc1": fc1_w.T.copy().astype(bf16),           # [C, FF]
        "wfc2": fc2_w.T.copy().astype(bf16),           # [FF, C]
        "sqb": sqb.astype(bf16),                       # [H, N, N]
        "n1w": np.asarray(inputs["norm1_w"], f32).reshape(KC, 128).T.copy(),
        "n1b": np.asarray(inputs["norm1_b"], f32).reshape(KC, 128).T.copy(),
        "n2w": np.asarray(inputs["norm2_w"], f32).reshape(KC, 128).T.copy(),
        "n2b": np.asarray(inputs["norm2_b"], f32).reshape(KC, 128).T.copy(),
        "projb": np.asarray(inputs["proj_b"], f32).reshape(1, C).astype(bf16),
        "fc1b": np.asarray(inputs["fc1_b"], f32).reshape(KFF, 128).T.copy(),
        "fc2b": np.asarray(inputs["fc2_b"], f32).reshape(1, C).astype(bf16),
    }
    in_maps = []
    for c in range(NCORES):
        xs = x[c * BLOC : (c + 1) * BLOC].reshape(T, C).T.copy()  # [C, T]
        in_maps.append({"x": xs, **common})
    return in_maps, one_m_sig


def build_bass(one_m_sig):
    nc = bass.Bass()
    dram = {}
    for name, shape, dt in [
        ("x", [C, T], F32),
        ("wqk", [C, 2 * CP], BF16),
        ("wv", [C, CP], BF16),
        ("wproj", [CP, C], BF16),
        ("wfc1", [C, FF], BF16),
        ("wfc2", [FF, C], BF16),
        ("sqb", [H, N, N], BF16),
        ("n1w", [128, KC], F32),
        ("n1b", [128, KC], F32),
        ("n2w", [128, KC], F32),
        ("n2b", [128, KC], F32),
        ("projb", [1, C], BF16),
        ("fc1b", [128, KFF], F32),
        ("fc2b", [1, C], BF16),
    ]:
        dram[name] = nc.declare_dram_parameter(name, shape, dt, isOutput=False)
    y_d = nc.declare_dram_parameter("y", [C, T], F32, isOutput=True)
    x1_d = nc.dram_tensor("x1s", [C, T], F32, kind="Internal")
    nc.stat1_d = nc.dram_tensor("stat1", [2, T], F32, kind="Internal")
    nc.stat2_d = nc.dram_tensor("stat2", [2, T], F32, kind="Internal")

    with PatchedTileContext(nc) as tc:
        build_body(nc, tc, dram, y_d, x1_d, one_m_sig)
    return nc


def _layernorm(nc, temps, stats_ps, out_pool, x_tiles, ones_col, w_sb, b_sb, tagp,
               stat_d, x_dram=None):
    """Feature-major layernorm over C. x source: 6 SBUF tiles [128, T] fp32,
    or (x_tiles=None) a DRAM tensor [C, T] streamed chunkwise.
    Returns 6 new bf16 tiles [128, T]."""
    out_tiles = [out_pool.tile([128, T], BF16, tag=f"{tagp}o{i}", name=f"{tagp}o{i}") for i in range(KC)]

    def get_x(ct, cs):
        if x_tiles is not None:
            return x_tiles[ct][:, cs]
        xt = temps.tile([128, TCH], F32, tag="lnx", name="lnx")
        nc.sync.dma_start(out=xt[:], in_=x_dram[bass.ts(ct, 128), cs])
        return xt[:]

    for ch in range(NCHUNK):
        cs = bass.ts(ch, TCH)
        s1 = stats_ps.tile([1, TCH], F32, tag="s1", name="s1")
        s2 = stats_ps.tile([1, TCH], F32, tag="s2", name="s2")
        for ct in range(KC):
            xs_ = get_x(ct, cs)
            x2t = temps.tile([128, TCH], BF16, tag="x2t", name="x2t")
            nc.scalar.activation(x2t[:], xs_, AF.Square)
            xb = temps.tile([128, TCH], BF16, tag="xb", name="xb")
            nc.any.tensor_copy(xb[:], xs_)
            nc.tensor.matmul(
                s1[:], ones_col[:], xb[:],
                start=(ct == 0), stop=(ct == KC - 1))
            nc.tensor.matmul(
                s2[:], ones_col[:], x2t[:],
                start=(ct == 0), stop=(ct == KC - 1))
        mu = temps.tile([1, TCH], F32, tag="mu", name="mu")
        nc.vector.tensor_scalar_mul(mu[:], s1[:], 1.0 / C)
        ex2 = temps.tile([1, TCH], F32, tag="ex2", name="ex2")
        nc.vector.tensor_scalar_mul(ex2[:], s2[:], 1.0 / C)
        mu2 = temps.tile([1, TCH], F32, tag="mu2", name="mu2")
        nc.vector.tensor_mul(mu2[:], mu[:], mu[:])
        var = temps.tile([1, TCH], F32, tag="var", name="var")
        nc.vector.tensor_sub(var[:], ex2[:], mu2[:])
        std = temps.tile([1, TCH], F32, tag="std", name="std")
        nc.scalar.activation(std[:], var[:], AF.Sqrt, bias=nc.consts_eps[:])
        rstd = temps.tile([1, TCH], F32, tag="rstd", name="rstd")
        nc.vector.reciprocal(rstd[:], std[:])
        nc.sync.dma_start(out=stat_d[0:1, cs], in_=mu[:])
        nc.sync.dma_start(out=stat_d[1:2, cs], in_=rstd[:])
        mu_b = temps.tile([128, TCH], F32, tag="mu_b", name="mu_b")
        nc.gpsimd.dma_start(out=mu_b[:], in_=stat_d[0:1, cs].to_broadcast((128, TCH)))
        rstd_b = temps.tile([128, TCH], F32, tag="rstd_b", name="rstd_b")
        nc.gpsimd.dma_start(out=rstd_b[:], in_=stat_d[1:2, cs].to_broadcast((128, TCH)))
        for ct in range(KC):
            t1 = temps.tile([128, TCH], F32, tag="t1", name="t1")
            nc.vector.tensor_sub(t1[:], get_x(ct, cs), mu_b[:])
            t2 = temps.tile([128, TCH], F32, tag="t2", name="t2")
            nc.vector.scalar_tensor_tensor(
                t2[:], t1[:], w_sb[:, ct : ct + 1], rstd_b[:],
                op0=ALU.mult, op1=ALU.mult)
            nc.scalar.activation(
                out_tiles[ct][:, cs], t2[:], AF.Identity,
                bias=b_sb[:, ct : ct + 1])
    return out_tiles


def build_body(nc, tc, dram, y_d, x1_d, one_m_sig):
    consts = tc.alloc_tile_pool(name="consts", bufs=1)
    temps = tc.alloc_tile_pool(name="temps", bufs=2)

    ident = consts.tile([128, 128], BF16, tag="ident", name="ident")
    make_identity(nc, ident[:])
    ones_col = consts.tile([128, 1], BF16, tag="ones_col", name="ones_col")
    nc.vector.memset(ones_col[:], 1.0)
    ones_row = consts.tile([1, TCH], BF16, tag="ones_row", name="ones_row")
    nc.vector.memset(ones_row[:], 1.0)
    eps_t = consts.tile([1, 1], F32, tag="eps_t", name="eps_t")
    nc.vector.memset(eps_t[:], EPS)
    nc.consts_eps = eps_t
    small = {}
    for nm in ("n1w", "n1b", "n2w", "n2b", "projb", "fc1b", "fc2b"):
        dt_ = BF16 if nm in ("projb", "fc2b") else F32
        t = consts.tile(list(dram[nm].shape), dt_, tag=nm, name=nm)
        nc.sync.dma_start(out=t[:], in_=dram[nm][:])
        small[nm] = t

    # ---------------- Phase 1: LN1 -> xn1 bf16 ----------------
    pool_xn1 = tc.alloc_tile_pool(name="xn1", bufs=1)
    pool_x = tc.alloc_tile_pool(name="x", bufs=1)
    x_tiles = []
    for i in range(KC):
        t = pool_x.tile([128, T], F32, tag=f"x{i}", name=f"x{i}")
        nc.sync.dma_start(out=t[:], in_=dram["x"][bass.ts(i, 128), :])
        x_tiles.append(t)
    stats_ps = tc.alloc_tile_pool(name="stats_ps", bufs=2, space="PSUM")
    xn1 = _layernorm(nc, temps, stats_ps, pool_xn1, x_tiles, ones_col,
                     small["n1w"], small["n1b"], "xn1", nc.stat1_d)
    stats_ps.release()
    pool_x.release()

    # weights for qk/v
    pool_wqk = tc.alloc_tile_pool(name="wqk", bufs=1)
    wqk_sb = []
    for i in range(KC):
        t = pool_wqk.tile([128, 2 * CP], BF16, tag=f"wqk{i}", name=f"wqk{i}")
        nc.sync.dma_start(out=t[:], in_=dram["wqk"][bass.ts(i, 128), :])
        wqk_sb.append(t)
    pool_wv = tc.alloc_tile_pool(name="wv", bufs=1)
    wv_sb = []
    for i in range(KC):
        t = pool_wv.tile([128, CP], BF16, tag=f"wv{i}", name=f"wv{i}")
        nc.sync.dma_start(out=t[:], in_=dram["wv"][bass.ts(i, 128), :])
        wv_sb.append(t)

    # ---------------- Phase 2: qk projection ----------------
    pool_qkv = tc.alloc_tile_pool(name="qkv", bufs=1, side="right")
    qk_sb = [pool_qkv.tile([128, T], BF16, tag=f"qk{m}", name=f"qk{m}") for m in range(2 * KCP)]
    ps_qk = tc.alloc_tile_pool(name="ps_qk", bufs=3, space="PSUM")
    for m in range(2 * KCP):
        for ch in range(NCHUNK):
            cs = bass.ts(ch, TCH)
            ps = ps_qk.tile([128, TCH], F32, tag="psqk", name="psqk")
            for k in range(KC):
                nc.tensor.matmul(
                    ps[:], wqk_sb[k][:, bass.ts(m, 128)], xn1[k][:, cs],
                    start=(k == 0), stop=(k == KC - 1))
            nc.any.tensor_copy(qk_sb[m][:, cs], ps[:])

    # ---------------- Phase 3: v projection (token-major, per batch) -------
    vT = [pool_qkv.tile([nn, CP], BF16, tag=f"vT{2 * b + i}", name=f"vT{2 * b + i}")
          for b in range(BLOC) for i, (no, nn) in enumerate(NT)]
    ps_v = tc.alloc_tile_pool(name="ps_v", bufs=3, space="PSUM")
    for b in range(BLOC):
        for half, (no, nn) in enumerate(NT):
            for nch in range(2):
                ps = ps_v.tile([128, 512], F32, tag="psv", name="psv")
                for k in range(KC):
                    nc.tensor.matmul(
                        ps[:nn], xn1[k][:, N * b + no : N * b + no + nn],
                        wv_sb[k][:, bass.ts(nch, 512)],
                        start=(k == 0), stop=(k == KC - 1))
                nc.any.tensor_copy(
                    vT[2 * b + half][:nn, bass.ts(nch, 512)], ps[:nn])
    ps_v.release()
    ps_qk.release()
    pool_wv.release()
    pool_wqk.release()
    pool_xn1.release()

    # proj weights + sqb
    pool_wproj = tc.alloc_tile_pool(name="wproj", bufs=1, side="right")
    wproj_sb = []
    for i in range(KCP):
        t = pool_wproj.tile([128, C], BF16, tag=f"wproj{i}", name=f"wproj{i}")
        nc.sync.dma_start(out=t[:], in_=dram["wproj"][bass.ts(i, 128), :])
        wproj_sb.append(t)
    pool_sqb = tc.alloc_tile_pool(name="sqb", bufs=1)
    sqb_sb = []
    for h in range(H):
        t1 = pool_sqb.tile([128, N], BF16, tag=f"sqb{h}_0", name=f"sqb{h}_0")
        nc.sync.dma_start(out=t1[:], in_=dram["sqb"][h, 0:128, :])
        t2 = pool_sqb.tile([68, N], BF16, tag=f"sqb{h}_1", name=f"sqb{h}_1")
        nc.sync.dma_start(out=t2[:], in_=dram["sqb"][h, 128:196, :])
        sqb_sb.append((t1, t2))

    # ------- Phase 4+5: attention + proj interleaved over batch pairs -------
    pool_O = tc.alloc_tile_pool(name="O", bufs=1, side="right")
    O_sb = [pool_O.tile([128, T], BF16, tag=f"O{i}", name=f"O{i}") for i in range(KCP)]
    attn_sm = tc.alloc_tile_pool(name="attn_sm", bufs=2)
    ps_sa = tc.alloc_tile_pool(name="ps_sa", bufs=2, space="PSUM")
    ps_sb = tc.alloc_tile_pool(name="ps_sb", bufs=1, space="PSUM")
    ps_t = tc.alloc_tile_pool(name="ps_t", bufs=2, space="PSUM")
    ps_o = tc.alloc_tile_pool(name="ps_o", bufs=1, space="PSUM")
    ps_p = tc.alloc_tile_pool(name="ps_p", bufs=2, space="PSUM")
    for g in range(NCHUNK):
        bb = 2 * g
        for h in range(H):
            qt = qk_sb[h // 2]
            kt = qk_sb[KCP + h // 2]
            ko = DP * (h % 2)
            sq1, sq2 = sqb_sb[h]
            oms = float(one_m_sig[h])
            psSa = ps_sa.tile([128, 2 * N], F32, tag="psSa", name="psSa")
            psSb = ps_sb.tile([68, 2 * N], F32, tag="psSb", name="psSb")
            for j in range(2):
                tb = N * (bb + j)
                nc.tensor.matmul(
                    psSa[:, N * j : N * j + N], qt[ko : ko + DP, tb : tb + 128],
                    kt[ko : ko + DP, tb : tb + N], start=True, stop=True)
                nc.tensor.matmul(
                    psSb[:, N * j : N * j + N], qt[ko : ko + DP, tb + 128 : tb + N],
                    kt[ko : ko + DP, tb : tb + N], start=True, stop=True)
            Ea = attn_sm.tile([128, 2 * N], BF16, tag="Ea", name="Ea")
            nc.scalar.activation(Ea[:], psSa[:], AF.Exp, scale=SCALE)
            Eb = attn_sm.tile([68, 2 * N], BF16, tag="Eb", name="Eb")
            nc.scalar.activation(Eb[:], psSb[:], AF.Exp, scale=SCALE)
            dra = attn_sm.tile([128, 2], F32, tag="dra", name="dra")
            drb = attn_sm.tile([68, 2], F32, tag="drb", name="drb")
            for j in range(2):
                js = bass.ds(N * j, N)
                nc.vector.tensor_reduce(
                    dra[:, j : j + 1], Ea[:, js], axis=mybir.AxisListType.X,
                    op=ALU.add)
                nc.vector.tensor_reduce(
                    drb[:, j : j + 1], Eb[:, js], axis=mybir.AxisListType.X,
                    op=ALU.add)
            ra = attn_sm.tile([128, 2], F32, tag="ra", name="ra")
            nc.vector.reciprocal(ra[:], dra[:])
            rb = attn_sm.tile([68, 2], F32, tag="rb", name="rb")
            nc.vector.reciprocal(rb[:], drb[:])
            Sa = attn_sm.tile([128, 2 * N], BF16, tag="Sa", name="Sa")
            Sb = attn_sm.tile([68, 2 * N], BF16, tag="Sb", name="Sb")
            for j in range(2):
                js = bass.ds(N * j, N)
                nc.vector.tensor_scalar(
                    Sa[:, js], Ea[:, js], ra[:, j : j + 1], None, op0=ALU.mult)
                nc.vector.tensor_scalar(
                    Sb[:, js], Eb[:, js], rb[:, j : j + 1], None, op0=ALU.mult)
            psO = ps_o.tile([DP, 2 * N], F32, tag="psO", name="psO")
            for j in range(2):
                js = bass.ds(N * j, N)
                jo = N * j
                psT = ps_t.tile([128, 2 * N], BF16, tag="psT", name="psT")
                nc.tensor.transpose(
                    psT[0:128, 0:128], Sa[:, jo : jo + 128], ident[:])
                nc.tensor.transpose(
                    psT[0:128, 128:196], Sb[:, jo : jo + 128],
                    ident[0:68, 0:68])
                nc.tensor.transpose(
                    psT[0:68, N : N + 128], Sa[:, jo + 128 : jo + N], ident[:])
                nc.tensor.transpose(
                    psT[0:68, N + 128 : 2 * N], Sb[:, jo + 128 : jo + N],
                    ident[0:68, 0:68])
                Sts1 = attn_sm.tile([128, N], BF16, tag="Sts1", name="Sts1")
                nc.any.tensor_tensor(Sts1[:], psT[:, 0:N], sq1[:], op=ALU.add)
                Sts2 = attn_sm.tile([68, N], BF16, tag="Sts2", name="Sts2")
                nc.any.tensor_tensor(
                    Sts2[:], psT[0:68, N : 2 * N], sq2[:], op=ALU.add)
                tb2 = 2 * (bb + j)
                nc.tensor.matmul(psO[:, js], vT[tb2][:, DP * h : DP * h + DP],
                                 Sts1[:], start=True, stop=False)
                nc.tensor.matmul(psO[:, js],
                                 vT[tb2 + 1][:68, DP * h : DP * h + DP],
                                 Sts2[:], start=False, stop=True)
            nc.scalar.activation(
                O_sb[h // 2][ko : ko + DP, N * bb : N * bb + 2 * N], psO[:],
                AF.Copy, scale=oms)
        # ---- proj + residual for this token chunk (pipelines with attn) ----
        cs = bass.ts(g, TCH)
        for m in range(KC):
            ps = ps_p.tile([128, TCH], F32, tag="psP", name="psP")
            nc.tensor.matmul(
                ps[:], small["projb"][:, bass.ts(m, 128)],
                ones_row[:], start=True, stop=False)
            for k in range(KCP):
                nc.tensor.matmul(
                    ps[:], wproj_sb[k][:, bass.ts(m, 128)], O_sb[k][:, cs],
                    start=False, stop=(k == KCP - 1))
            xres = temps.tile([128, TCH], F32, tag="xres", name="xres")
            nc.sync.dma_start(out=xres[:], in_=dram["x"][bass.ts(m, 128), cs])
            x1t = temps.tile([128, TCH], F32, tag="x1t", name="x1t")
            nc.vector.tensor_add(x1t[:], ps[:], xres[:])
            nc.sync.dma_start(out=x1_d[bass.ts(m, 128), cs], in_=x1t[:])
    ps_p.release()
    ps_o.release()
    ps_t.release()
    ps_sb.release()
    ps_sa.release()
    attn_sm.release()
    pool_sqb.release()
    pool_O.release()
    pool_wproj.release()
    pool_qkv.release()

    # fc weights
    pool_wfc = tc.alloc_tile_pool(name="wfc", bufs=1, side="right")
    wfc1_sb = []
    for i in range(KC):
        t = pool_wfc.tile([128, FF], BF16, tag=f"wfc1_{i}", name=f"wfc1_{i}")
        nc.sync.dma_start(out=t[:], in_=dram["wfc1"][bass.ts(i, 128), :])
        wfc1_sb.append(t)
    wfc2_sb = []
    for i in range(KFF):
        t = pool_wfc.tile([128, C], BF16, tag=f"wfc2_{i}", name=f"wfc2_{i}")
        nc.sync.dma_start(out=t[:], in_=dram["wfc2"][bass.ts(i, 128), :])
        wfc2_sb.append(t)

    # ---------------- Phase 6: LN2 -> xn2 bf16 ----------------
    pool_xn2 = tc.alloc_tile_pool(name="xn2", bufs=1, side="right")
    stats_ps2 = tc.alloc_tile_pool(name="stats_ps2", bufs=2, space="PSUM")
    xn2 = _layernorm(nc, temps, stats_ps2, pool_xn2, None, ones_col,
                     small["n2w"], small["n2b"], "xn2", nc.stat2_d, x_dram=x1_d)
    stats_ps2.release()

    # ---------------- Phase 7: MLP + residual -> y ----------------
    pool_hdn = tc.alloc_tile_pool(name="hdn", bufs=2)
    ps_f1 = tc.alloc_tile_pool(name="ps_f1", bufs=2, space="PSUM")
    ps_f2 = tc.alloc_tile_pool(name="ps_f2", bufs=2, space="PSUM")
    for ch in range(NCHUNK):
        cs = bass.ts(ch, TCH)
        hdn = [pool_hdn.tile([128, TCH], BF16, tag=f"hdn{m}", name=f"hdn{m}") for m in range(KFF)]
        for m in range(KFF):
            ps = ps_f1.tile([128, TCH], F32, tag="psF1", name="psF1")
            for k in range(KC):
                nc.tensor.matmul(
                    ps[:], wfc1_sb[k][:, bass.ts(m, 128)], xn2[k][:, cs],
                    start=(k == 0), stop=(k == KC - 1))
            nc.scalar.activation(hdn[m][:], ps[:], AF.Gelu,
                                 bias=small["fc1b"][:, m : m + 1])
        for m in range(KC):
            ps = ps_f2.tile([128, TCH], F32, tag="psF2", name="psF2")
            nc.tensor.matmul(
                ps[:], small["fc2b"][:, bass.ts(m, 128)],
                ones_row[:], start=True, stop=False)
            for k in range(KFF):
                nc.tensor.matmul(
                    ps[:], wfc2_sb[k][:, bass.ts(m, 128)], hdn[k][:],
                    start=False, stop=(k == KFF - 1))
            x1res = temps.tile([128, TCH], F32, tag="x1res", name="x1res")
            nc.sync.dma_start(out=x1res[:], in_=x1_d[bass.ts(m, 128), cs])
            ych = temps.tile([128, TCH], F32, tag="ych", name="ych")
            nc.vector.tensor_add(ych[:], ps[:], x1res[:])
            nc.sync.dma_start(out=y_d[bass.ts(m, 128), cs], in_=ych[:])
    ps_f2.release()
    ps_f1.release()
    pool_hdn.release()
    pool_xn2.release()
    pool_wfc.release()
    temps.release()
    consts.release()


def postprocess(results):
    """results: list of per-core out dicts with y [C, T] -> full [B, N, C]."""
    outs = []
    for c in range(NCORES):
        y = np.asarray(results[c]["y"])  # [C, T]
        outs.append(y.T.reshape(BLOC, N, C))
    return np.concatenate(outs, 0)


# ----------------------------------------------------------------------------
# Entry point: FULL inputs -> FULL output (8-core SPMD data-parallel).
# ----------------------------------------------------------------------------
_BUILD_CACHE = {}
LAST_RESULT = None


def kernel(**inputs) -> np.ndarray:
    global LAST_RESULT
    import os

    trace = os.environ.get("KERNEL_TRACE", "0") == "1"
    if trace:
        _install_ntff_shim()
    else:
        os.environ.setdefault("BASS_NEVER_TRACE", "1")
    from concourse.bass_utils import run_bass_kernel_spmd

    in_maps, oms = host_prep(inputs)
    key = tuple(np.asarray(oms, np.float64).tolist())
    nc = _BUILD_CACHE.get(key)
    if nc is None:
        nc = build_bass(oms)
        _BUILD_CACHE[key] = nc
    kw = {}
    if trace:
        kw = dict(trace=True, tmpdir=os.environ.get("KERNEL_TRACE_DIR", None))
    res = run_bass_kernel_spmd(nc, in_maps, list(range(NCORES)), **kw)
    LAST_RESULT = res
    return postprocess(res.results)


def _install_ntff_shim():
    """Register the NTFF profile hook that this image's antenv lacks."""
    import types

    import antenv
    from concourse import bass_utils

    bass_utils.upload_artifacts = lambda tmpdir: f"local:{tmpdir}"
    if "antenv.axon_hooks" in sys.modules:
        return
    mod = types.ModuleType("antenv.axon_hooks")
    mod._hook = None
    mod.set_axon_ntff_profile_hook = lambda hook: setattr(mod, "_hook", hook)
    mod.get_axon_ntff_profile_hook = lambda: mod._hook
    sys.modules["antenv.axon_hooks"] = mod
    antenv.axon_hooks = mod
    from trn_agent_boot.trn_boot import _ntff_profile_via_ctypes

    hook = _ntff_profile_via_ctypes("/opt/axon/libaxon_pjrt.so")
    if hook is not None:
        mod.set_axon_ntff_profile_hook(hook)

